# revision 10
# baseline (speedup 1.0000x reference)
"""MixerBlock TRN2 kernel: B=2, S=4096, E=1024, DF=4096 on 8 NeuronCores.

Strategy (two SPMD launches):
  Phase 1 (shard B*S=8192 rows -> 1024 rows/core):
    h   = LN(x)            (cn affine folded into W1/b1 host-side)
    a   = silu(h @ W1g + b1')        -> kept transposed aT[df, tok]
    y   = x + aT.T @ W2 + b2
    h2  = LN(y)*tn_g + tn_b          (bf16)
    outputs y (f32), h2 (bf16)
  Phase 2 (shard E=1024 -> 128 channels/core; rows (b,e) = 256/core):
    out[be, s] = sum_t h2T[t, be] * M[t, s] + tb[s] + y[be, s]
    The Toeplitz matrix M[t,s] = tw[s-t] (s>=t) is diagonal-constant, so a
    [128t x 512s] tile depends only on (512*sb - 128*t): 32 distinct tiles,
    prebuilt host-side from tw (4 MB bf16), used as the moving operand.
"""

import os
import sys

sys.path.insert(0, "/opt/trn_rl_repo")
sys.path.insert(0, "/opt/trn_rl_repo/concourse")

import numpy as np
import ml_dtypes

import concourse.bass as bass
import concourse.bacc as bacc
import concourse.mybir as mybir
from concourse import tile
from concourse import bass_utils
from concourse.bass_interp import get_hw_module

dt = mybir.dt
AF = mybir.ActivationFunctionType
AX = mybir.AxisListType
BF16 = ml_dtypes.bfloat16

B, S, E = 2, 4096, 1024
DF = 4 * E
EPS = 1e-5
NCORES = 8
RPC = (B * S) // NCORES      # 1024 rows per core (phase 1)
EPC = E // NCORES            # 128 channels per core (phase 2)
BE = B * EPC                 # 256 (b,e) rows per core (phase 2)

LAST_TIMINGS = {}

# --------------------------------------------------------------------------
# phase 1 program
# --------------------------------------------------------------------------


def build_phase1():
    nc = bacc.Bacc("TRN2", target_bir_lowering=False, debug=False,
                   enable_asserts=False, num_devices=NCORES)
    x_d = nc.dram_tensor("x", [RPC, E], dt.float32, kind="ExternalInput").ap()
    w1_d = nc.dram_tensor("w1", [E, DF], dt.bfloat16, kind="ExternalInput").ap()
    w2_d = nc.dram_tensor("w2", [DF, E], dt.bfloat16, kind="ExternalInput").ap()
    b1_d = nc.dram_tensor("b1", [128, 32], dt.float32, kind="ExternalInput").ap()
    b2_d = nc.dram_tensor("b2", [128, E], dt.float32, kind="ExternalInput").ap()
    tng_d = nc.dram_tensor("tng", [128, E], dt.bfloat16, kind="ExternalInput").ap()
    tnb_d = nc.dram_tensor("tnb", [128, E], dt.bfloat16, kind="ExternalInput").ap()
    id_d = nc.dram_tensor("ident", [128, 128], dt.bfloat16, kind="ExternalInput").ap()
    y_d = nc.dram_tensor("y", [RPC, E], dt.float32, kind="ExternalOutput").ap()
    h2_d = nc.dram_tensor("h2", [RPC, E], dt.bfloat16, kind="ExternalOutput").ap()

    NT = 4          # token tiles per block (block = 512 tokens)
    NBLK = RPC // (128 * NT)   # 2 blocks

    with tile.TileContext(nc) as tc:
        with (
            tc.tile_pool(name="const", bufs=1) as constp,
            tc.tile_pool(name="w1p", bufs=8) as w1p,
            tc.tile_pool(name="xp", bufs=5) as xp,
            tc.tile_pool(name="xcp", bufs=2) as xcp,
            tc.tile_pool(name="sqp", bufs=2) as sqp,
            tc.tile_pool(name="stat", bufs=24) as statp,
            tc.tile_pool(name="hbf", bufs=5) as hbfp,
            tc.tile_pool(name="htp", bufs=9) as htp,
            tc.tile_pool(name="atp", bufs=34) as atp,
            tc.tile_pool(name="w2p", bufs=6) as w2p,
            tc.tile_pool(name="yp", bufs=4) as yp,
            tc.tile_pool(name="zp", bufs=3) as zp,
            tc.tile_pool(name="h2p", bufs=3) as h2p,
            tc.tile_pool(name="tps", bufs=2, space="PSUM") as tpsum,
            tc.tile_pool(name="m1ps", bufs=2, space="PSUM") as m1psum,
            tc.tile_pool(name="m2ps", bufs=4, space="PSUM") as m2psum,
        ):
            # resident constants
            w1_sb = []
            for i in range(8):
                t = w1p.tile([128, DF], dt.bfloat16, tag="w1sb")
                nc.sync.dma_start(out=t[:, :], in_=w1_d[i * 128:(i + 1) * 128, :])
                w1_sb.append(t)
            b1_sb = constp.tile([128, 32], dt.float32, tag="b1")
            nc.sync.dma_start(out=b1_sb[:, :], in_=b1_d[:, :])
            b2_sb = constp.tile([128, E], dt.float32, tag="b2")
            nc.sync.dma_start(out=b2_sb[:, :], in_=b2_d[:, :])
            tng_sb = constp.tile([128, E], dt.bfloat16, tag="tng")
            nc.sync.dma_start(out=tng_sb[:, :], in_=tng_d[:, :])
            tnb_sb = constp.tile([128, E], dt.bfloat16, tag="tnb")
            nc.sync.dma_start(out=tnb_sb[:, :], in_=tnb_d[:, :])
            id_sb = constp.tile([128, 128], dt.bfloat16, tag="ident")
            nc.sync.dma_start(out=id_sb[:, :], in_=id_d[:, :])
            eps_sb = constp.tile([128, 1], dt.float32, tag="eps")
            nc.gpsimd.memset(eps_sb[:, :], EPS)

            def layernorm_to(src, dst_bf, scale_rows=None, bias_rows=None):
                """dst_bf (bf16) = LN(src) [* scale_rows + bias_rows]."""
                ssum = statp.tile([128, 1], dt.float32, tag="ssum")
                nc.vector.reduce_sum(ssum[:, :], src[:, :], axis=AX.X)
                negmean = statp.tile([128, 1], dt.float32, tag="negmean")
                nc.scalar.mul(negmean[:, :], ssum[:, :], -1.0 / E)
                xc = xcp.tile([128, E], dt.float32, tag="xc")
                nc.vector.tensor_scalar_add(xc[:, :], src[:, :], negmean[:, :])
                sq = sqp.tile([128, E], dt.bfloat16, tag="sq")
                ssq = statp.tile([128, 1], dt.float32, tag="ssq")
                nc.scalar.activation(sq[:, :], xc[:, :], AF.Square,
                                     accum_out=ssq[:, :])
                std = statp.tile([128, 1], dt.float32, tag="std")
                nc.scalar.activation(std[:, :], ssq[:, :], AF.Sqrt,
                                     scale=1.0 / E, bias=eps_sb[:, :])
                rstd = statp.tile([128, 1], dt.float32, tag="rstd")
                nc.vector.reciprocal(rstd[:, :], std[:, :])
                nc.scalar.activation(dst_bf[:, :], xc[:, :], AF.Copy,
                                     scale=rstd[:, :])
                if scale_rows is not None:
                    nc.vector.tensor_mul(dst_bf[:, :], dst_bf[:, :],
                                         scale_rows[:, :])
                if bias_rows is not None:
                    nc.vector.tensor_add(dst_bf[:, :], dst_bf[:, :],
                                         bias_rows[:, :])

            for blk in range(NBLK):
                row0 = blk * 128 * NT
                # ---- load x, LN1 -> hbf (bf16, [tok, e]) ----
                x_t, hbf_t = [], []
                for tt in range(NT):
                    xt = xp.tile([128, E], dt.float32, tag="xt")
                    nc.sync.dma_start(
                        out=xt[:, :],
                        in_=x_d[row0 + tt * 128: row0 + (tt + 1) * 128, :])
                    x_t.append(xt)
                for tt in range(NT):
                    hb = hbfp.tile([128, E], dt.bfloat16, tag="hb")
                    layernorm_to(x_t[tt], hb)
                    hbf_t.append(hb)
                # ---- transpose h -> hT[e_tile][e 128, tok 512] ----
                hT = []
                for e in range(8):
                    pt = tpsum.tile([128, 512], dt.bfloat16, tag="tp")
                    for tt in range(NT):
                        nc.tensor.transpose(
                            pt[:, tt * 128:(tt + 1) * 128],
                            hbf_t[tt][:, e * 128:(e + 1) * 128],
                            id_sb[:, :])
                    ht = htp.tile([128, 512], dt.bfloat16, tag="ht")
                    nc.vector.tensor_copy(ht[:, :], pt[:, :])
                    hT.append(ht)
                # ---- mm1 + silu -> aT[df][df 128, tok 512] (bf16) ----
                aT = []
                for df in range(32):
                    ps = m1psum.tile([128, 512], dt.float32, tag="m1")
                    for e in range(8):
                        nc.tensor.matmul(
                            ps[:, :],
                            w1_sb[e][:, df * 128:(df + 1) * 128],
                            hT[e][:, :],
                            start=(e == 0), stop=(e == 7))
                    at = atp.tile([128, 512], dt.bfloat16, tag="at")
                    nc.scalar.activation(at[:, :], ps[:, :], AF.Silu,
                                         bias=b1_sb[:, df:df + 1])
                    aT.append(at)
                # ---- mm2 (stream W2) -> y = x + out + b2 ----
                y_t = [yp.tile([128, E], dt.float32, tag="yt", name=f"yt{blk}_{i}")
                       for i in range(NT)]
                for eb in range(2):
                    pss = [m2psum.tile([128, 512], dt.float32, tag="m2",
                                       name=f"m2_{blk}_{eb}_{i}")
                           for i in range(NT)]
                    for df in range(32):
                        w2t = w2p.tile([128, 512], dt.bfloat16, tag="w2t")
                        nc.sync.dma_start(
                            out=w2t[:, :],
                            in_=w2_d[df * 128:(df + 1) * 128,
                                     eb * 512:(eb + 1) * 512])
                        for tt in range(NT):
                            nc.tensor.matmul(
                                pss[tt][:, :],
                                aT[df][:, tt * 128:(tt + 1) * 128],
                                w2t[:, :],
                                start=(df == 0), stop=(df == 31))
                    for tt in range(NT):
                        ysl = y_t[tt][:, eb * 512:(eb + 1) * 512]
                        nc.vector.tensor_add(
                            ysl, pss[tt][:, :],
                            x_t[tt][:, eb * 512:(eb + 1) * 512])
                        nc.gpsimd.tensor_add(
                            ysl, ysl, b2_sb[:, eb * 512:(eb + 1) * 512])
                # ---- write y, LN2 -> h2 ----
                for tt in range(NT):
                    nc.sync.dma_start(
                        out=y_d[row0 + tt * 128: row0 + (tt + 1) * 128, :],
                        in_=y_t[tt][:, :])
                    h2t = h2p.tile([128, E], dt.bfloat16, tag="h2t")
                    layernorm_to(y_t[tt], h2t, scale_rows=tng_sb,
                                 bias_rows=tnb_sb)
                    nc.sync.dma_start(
                        out=h2_d[row0 + tt * 128: row0 + (tt + 1) * 128, :],
                        in_=h2t[:, :])
    nc.compile()
    nc.m = get_hw_module(nc.m)
    return nc


# --------------------------------------------------------------------------
# phase 2 program
# --------------------------------------------------------------------------


def build_phase2():
    nc = bacc.Bacc("TRN2", target_bir_lowering=False, debug=False,
                   enable_asserts=False, num_devices=NCORES)
    h2t_d = nc.dram_tensor("h2t", [S, BE], dt.bfloat16, kind="ExternalInput").ap()
    r_d = nc.dram_tensor("rt", [S, 512], dt.bfloat16, kind="ExternalInput").ap()
    yt_d = nc.dram_tensor("yt", [BE, S], dt.float32, kind="ExternalInput").ap()
    tb_d = nc.dram_tensor("tb", [1, S], dt.bfloat16, kind="ExternalInput").ap()
    ones_d = nc.dram_tensor("ones", [1, 128], dt.bfloat16, kind="ExternalInput").ap()
    out_d = nc.dram_tensor("out", [BE, S], dt.float32, kind="ExternalOutput").ap()

    with tile.TileContext(nc) as tc:
        with (
            tc.tile_pool(name="hs", bufs=32) as hsp,
            tc.tile_pool(name="rt", bufs=32) as rtp,
            tc.tile_pool(name="const", bufs=1) as constp,
            tc.tile_pool(name="yin", bufs=6) as yinp,
            tc.tile_pool(name="outp", bufs=6) as outp,
            tc.tile_pool(name="ps", bufs=8, space="PSUM") as psp,
        ):
            hs, rt = [], []
            for t in range(32):
                h = hsp.tile([128, BE], dt.bfloat16, tag="hs")
                nc.sync.dma_start(out=h[:, :],
                                  in_=h2t_d[t * 128:(t + 1) * 128, :])
                hs.append(h)
            for d in range(32):
                r = rtp.tile([128, 512], dt.bfloat16, tag="rt")
                nc.sync.dma_start(out=r[:, :],
                                  in_=r_d[d * 128:(d + 1) * 128, :])
                rt.append(r)
            tb_sb = constp.tile([1, S], dt.bfloat16, tag="tb")
            nc.sync.dma_start(out=tb_sb[:, :], in_=tb_d[:, :])
            ones_sb = constp.tile([1, 128], dt.bfloat16, tag="ones")
            nc.sync.dma_start(out=ones_sb[:, :], in_=ones_d[:, :])

            for be in range(2):
                ps = [psp.tile([128, 512], dt.float32, tag="ps", name=f"ps{be}_{i}")
                      for i in range(8)]
                lhs_col = slice(be * 128, (be + 1) * 128)
                for t in range(32):
                    for sb in range(t // 4, 8):
                        d = 4 * sb - t + 3
                        nc.tensor.matmul(
                            ps[sb][:, :], hs[t][:, lhs_col], rt[d][:, :],
                            start=(t == 0), stop=False)
                    # drain any s_block whose accumulation just completed
                    for sb in range(8):
                        if t == 4 * sb + 3:
                            nc.tensor.matmul(
                                ps[sb][:, :], ones_sb[:, :],
                                tb_sb[:, sb * 512:(sb + 1) * 512],
                                start=False, stop=True)
                            yin = yinp.tile([128, 512], dt.float32, tag="yin")
                            nc.sync.dma_start(
                                out=yin[:, :],
                                in_=yt_d[be * 128:(be + 1) * 128,
                                         sb * 512:(sb + 1) * 512])
                            ot = outp.tile([128, 512], dt.float32, tag="ot")
                            nc.vector.tensor_add(ot[:, :], ps[sb][:, :],
                                                 yin[:, :])
                            nc.sync.dma_start(
                                out=out_d[be * 128:(be + 1) * 128,
                                          sb * 512:(sb + 1) * 512],
                                in_=ot[:, :])
    nc.compile()
    nc.m = get_hw_module(nc.m)
    return nc


def _install_ntff_hook():
    """The agent image's antenv lacks axon_hooks; synthesize it so
    run_bass_kernel_spmd(trace=True) can capture NTFF profiles."""
    import types
    import antenv

    if "antenv.axon_hooks" in sys.modules:
        return
    mod = types.ModuleType("antenv.axon_hooks")
    state = {"h": None}
    mod.set_axon_ntff_profile_hook = lambda h: state.__setitem__("h", h)
    mod.get_axon_ntff_profile_hook = lambda: state["h"]
    sys.modules["antenv.axon_hooks"] = mod
    antenv.axon_hooks = mod
    from trn_agent_boot.trn_boot import _ntff_profile_via_ctypes

    mod.set_axon_ntff_profile_hook(
        _ntff_profile_via_ctypes("/opt/axon/libaxon_pjrt.so"))
    bass_utils.upload_artifacts = lambda tmpdir: tmpdir


_P1 = None
_P2 = None


def _programs():
    global _P1, _P2
    if _P1 is None:
        _P1 = build_phase1()
    if _P2 is None:
        _P2 = build_phase2()
    return _P1, _P2


def _run(nc, in_maps, trace):
    if trace:
        try:
            _install_ntff_hook()
        except Exception as e:
            print(f"ntff hook install failed: {e}", file=sys.stderr)
            trace = False
    res = bass_utils.run_bass_kernel_spmd(
        nc, in_maps, core_ids=list(range(NCORES)), trace=trace)
    return res


def kernel(x, cn_g, cn_b, W1, b1, W2, b2, tn_g, tn_b, tw, tb):
    trace = os.environ.get("MIXER_TRACE", "0") == "1"
    x = np.asarray(x, np.float32)
    p1, p2 = _programs()

    # ---- host prep (inputs only) ----
    W1 = np.asarray(W1, np.float32)
    W2 = np.asarray(W2, np.float32)
    cn_g = np.asarray(cn_g, np.float32)
    cn_b = np.asarray(cn_b, np.float32)
    w1g = (cn_g[:, None] * W1).astype(BF16)
    b1f = (np.asarray(b1, np.float32) + cn_b @ W1).astype(np.float32)
    b1_t = np.ascontiguousarray(b1f.reshape(32, 128).T)          # [128, 32]
    w2bf = W2.astype(BF16)
    b2b = np.ascontiguousarray(
        np.broadcast_to(np.asarray(b2, np.float32), (128, E)))
    tngb = np.ascontiguousarray(
        np.broadcast_to(np.asarray(tn_g, np.float32).astype(BF16), (128, E)))
    tnbb = np.ascontiguousarray(
        np.broadcast_to(np.asarray(tn_b, np.float32).astype(BF16), (128, E)))
    ident = np.eye(128, dtype=BF16)

    xf = x.reshape(B * S, E)
    in_maps1 = []
    for c in range(NCORES):
        in_maps1.append({
            "x": np.ascontiguousarray(xf[c * RPC:(c + 1) * RPC]),
            "w1": w1g, "w2": w2bf, "b1": b1_t, "b2": b2b,
            "tng": tngb, "tnb": tnbb, "ident": ident,
        })
    r1 = _run(p1, in_maps1, trace)
    if trace:
        LAST_TIMINGS["phase1_ns"] = r1.exec_time_ns
    y = np.concatenate([np.asarray(r1.results[c]["y"], np.float32)
                        for c in range(NCORES)], axis=0)
    h2 = np.concatenate([np.asarray(r1.results[c]["h2"]).view(BF16)
                         if r1.results[c]["h2"].dtype != BF16
                         else r1.results[c]["h2"]
                         for c in range(NCORES)], axis=0)

    # ---- phase 2 host glue ----
    tw = np.asarray(tw, np.float32)
    pad = np.zeros(512 + S + 512, np.float32)
    pad[512:512 + S] = tw
    # R[d][i, j] = tw_ext[(d-3)*128 + j - i]
    win = np.lib.stride_tricks.sliding_window_view(pad, 512)   # win[k] = pad[k:k+512]
    rtiles = np.empty((32, 128, 512), np.float32)
    ii = np.arange(128)
    for d in range(32):
        rtiles[d] = win[512 + (d - 3) * 128 - ii]
    rtiles_bf = rtiles.astype(BF16).reshape(S, 512)
    tb_row = np.asarray(tb, np.float32).astype(BF16).reshape(1, S)
    ones_row = np.ones((1, 128), BF16)

    h2v = h2.reshape(B, S, E)
    yv = y.reshape(B, S, E)
    in_maps2 = []
    for c in range(NCORES):
        e0 = c * EPC
        h2sl = np.ascontiguousarray(
            h2v[:, :, e0:e0 + EPC].transpose(1, 0, 2).reshape(S, BE))
        ysl = np.ascontiguousarray(
            yv[:, :, e0:e0 + EPC].transpose(0, 2, 1).reshape(BE, S))
        in_maps2.append({"h2t": h2sl, "rt": rtiles_bf, "yt": ysl,
                         "tb": tb_row, "ones": ones_row})
    r2 = _run(p2, in_maps2, trace)
    if trace:
        LAST_TIMINGS["phase2_ns"] = r2.exec_time_ns

    out = np.empty((B, S, E), np.float32)
    for c in range(NCORES):
        e0 = c * EPC
        o = np.asarray(r2.results[c]["out"], np.float32).reshape(B, EPC, S)
        out[:, :, e0:e0 + EPC] = o.transpose(0, 2, 1)
    return out
